# revision 8
# baseline (speedup 1.0000x reference)
"""Trainium2 Bass kernel for ClusterContrastiveLoss (N=65536, K=256).

Data-parallel over the batch axis: each of the 8 cores processes 8192 rows of
q/q_a, computing row-softmax and accumulating the K x K Gram matrices
    G_aa = qs^T @ qs,  G_ab = qs^T @ qas,  G_bb = qas^T @ qas
Since G_aa/G_bb are symmetric, only their upper block-triangles are computed
(4 matmuls per 128-row chunk with free dims 512/384/256/128 instead of
512/512/256/256).  Column marginals come for free on the host: softmax rows
sum to 1, so colsum(qs)[k] = sum_j G_aa[k, j].

The kernel is DMA-bound (16 MB/core of fp32 input ~= 50us at the ~340 GB/s
per-core HBM rate), so the softmax element work is spread over THREE engines,
each kept under that window.  Hardware-measured rates for the [128, 256]
per-(chunk, tensor) scale (multiply by a per-partition 1/rowsum):
    DVE  tensor_scalar (AP scalar)   281 ns   (2x_1p mode + Ptr overhead)
    ACT  scalar.mul    (AP scale)    583 ns
    Pool normalize_recip (custom Q7) 424 ns   (fused reciprocal + divide)
tensor_reduce only has a 1x uop (~1.07 ns/elem), so the rowsum reduce alone
costs ~35us of DVE -- hence the scale work mostly lives on Pool/ACT.
Per-half-superchunk (4 chunks) assignment:
    Pool halves: ACT exp -> f32 staging, DVE reduce, Pool normalize -> bf16
    DVE/ACT halves: ACT exp -> bf16, DVE reduce, DVE-or-ACT in-place scale
The first/last superchunks run chunk-at-a-time (fine DMA + B=1 exp) to
collapse the pipeline ramp/tail; dummy exp / normalize ops at kernel start
hoist the ACT table load (~1.3us) and the Q7 library load (~6us) under the
first DMA wait.  q DMAs issue from sync, q_a from gpsimd SWDGE so descriptor
issue never starves the 16 DMA engines.  Host sums per-core partials and
evaluates the closed-form loss on the tiny K x K matrices in float64.
"""

import numpy as np

N_TOTAL = 65536
K = 256
N_CORES = 8
SHARD = N_TOTAL // N_CORES  # 8192 rows per core
CHUNK_P = 128               # rows per compute chunk (SBUF partition dim)
SUPER = 8                   # chunks per DMA superchunk (1 MB per tensor)
EPS = 1e-8
LARGE_NUM = 1e9

_CACHE = {}

# Test-harness knobs (ignored in normal use): set _TRACE=True before calling
# kernel() to capture an NTFF profile; the BassKernelResults lands in _LAST.
_TRACE = False
_LAST = None

# Per-(superchunk, half) scale-engine assignment; tuned so each engine's
# total work stays under the ~50us DMA window (see module docstring).
ASSIGN = {
    0: ("DVE", "ACT"),
    1: ("POOL", "POOL"),
    2: ("POOL", "DVE"),
    3: ("POOL", "ACT"),
    4: ("POOL", "POOL"),
    5: ("POOL", "ACT"),
    6: ("POOL", "POOL"),
    7: ("POOL", "DVE"),
}


def _build(shard_rows):
    from contextlib import ExitStack

    import concourse.bass as bass  # noqa: F401
    import concourse.tile as tile
    from concourse import bacc, mybir

    n_chunks = shard_rows // CHUNK_P
    sc = min(SUPER, n_chunks)      # chunks per superchunk
    n_super = n_chunks // sc
    H = sc // 2                    # chunks per half-superchunk

    f32 = mybir.dt.float32
    bf16 = mybir.dt.bfloat16
    Exp = mybir.ActivationFunctionType.Exp
    Add = mybir.AluOpType.add
    X = mybir.AxisListType.X

    nc = bacc.Bacc("TRN2", target_bir_lowering=False, debug=False)
    q_ap = nc.dram_tensor(
        "q", [n_chunks, CHUNK_P, K], f32, kind="ExternalInput"
    ).ap()
    qa_ap = nc.dram_tensor(
        "q_a", [n_chunks, CHUNK_P, K], f32, kind="ExternalInput"
    ).ap()
    out_ap = nc.dram_tensor(
        "partials", [CHUNK_P, 10 * 128], f32, kind="ExternalOutput"
    ).ap()

    with tile.TileContext(nc) as tc, ExitStack() as ctx:
        inp = ctx.enter_context(tc.tile_pool(name="inp", bufs=4))
        work = ctx.enter_context(tc.tile_pool(name="work", bufs=4))
        stage = ctx.enter_context(tc.tile_pool(name="stage", bufs=3))
        stats = ctx.enter_context(tc.tile_pool(name="stats", bufs=4))
        psum = ctx.enter_context(tc.tile_pool(name="psum", bufs=1, space="PSUM"))
        outp = ctx.enter_context(tc.tile_pool(name="outp", bufs=1))

        # Accumulators, one PSUM bank each, live across the whole kernel.
        # Missing blocks are transposes of computed ones (host reconstructs):
        #   ps_a = [G_aa[0:128, :]   | G_ab[0:128, :]]   N=512
        #   ps_b = [G_aa[128:, 128:] | G_ab[128:, :]]    N=384
        #   ps_c =  G_bb[0:128, :]                       N=256
        #   ps_d =  G_bb[128:, 128:]                     N=128
        ps = [
            psum.tile([128, 512], f32, name="ps_a"),
            psum.tile([128, 384], f32, name="ps_b"),
            psum.tile([128, 256], f32, name="ps_c"),
            psum.tile([128, 128], f32, name="ps_d"),
        ]
        zbias = stats.tile([128, 1], f32, name="zbias", bufs=1)
        nc.vector.memset(zbias[:], 0.0)
        one = stats.tile([128, 1], f32, name="one", bufs=1)
        nc.vector.memset(one[:], 1.0)
        # Dummy ops: force the ACT table load (~1.3us) and the Pool Q7
        # library load (~6us) to happen under the first DMA wait instead of
        # stalling the first real activation / normalize.
        warm_a = stats.tile([128, 1], bf16, name="warm_a", bufs=1)
        nc.scalar.activation(warm_a[:], zbias[:], Exp, bias=zbias[:])
        warm_p = stats.tile([128, 1], bf16, name="warm_p", bufs=1)
        nc.gpsimd.normalize_recip(warm_p[:], zbias[:], one[:])

        def emit_chunk_matmuls(rhs, it):
            first = it == 0
            last = it == n_chunks - 1
            nc.tensor.matmul(ps[2][:], rhs[:, 256:384], rhs[:, 256:512],
                             start=first, stop=last)
            nc.tensor.matmul(ps[3][:], rhs[:, 384:512], rhs[:, 384:512],
                             start=first, stop=last)
            nc.tensor.matmul(ps[0][:], rhs[:, 0:128], rhs[:, 0:512],
                             start=first, stop=last)
            nc.tensor.matmul(ps[1][:], rhs[:, 128:256], rhs[:, 128:512],
                             start=first, stop=last)

        for s in range(n_super):
            fine = s == 0 or s == n_super - 1
            # Interleaved layout: qe[:, j, 0, :] = q chunk, qe[:, j, 1, :] =
            # q_a chunk, so each chunk's scaled [qs | qas] in ebf is a
            # contiguous [128, 512] whose slices serve as both lhsT and rhs.
            qe = inp.tile([128, sc, 2, K], f32, name="qe")
            ebf = work.tile([128, sc, 2 * K], bf16, name="ebf")
            acc = stats.tile([128, sc, 2], f32, name="acc")
            rt = stats.tile([128, sc, 2], f32, name="rt")

            if fine:
                # Ramp/tail: per-chunk DMAs so the first exp starts after
                # ~256KB (not 2MB) and the tail dependency chain is short.
                for j in range(sc):
                    nc.sync.dma_start(
                        qe[:, j : j + 1, 0, :],
                        q_ap[s * sc + j : s * sc + j + 1].rearrange(
                            "j p d -> p j d"),
                    )
                    nc.gpsimd.dma_start(
                        qe[:, j : j + 1, 1, :],
                        qa_ap[s * sc + j : s * sc + j + 1].rearrange(
                            "j p d -> p j d"),
                    )
            else:
                nc.sync.dma_start(
                    qe[:, :, 0, :],
                    q_ap[s * sc : (s + 1) * sc].rearrange("j p d -> p j d"),
                )
                nc.gpsimd.dma_start(
                    qe[:, :, 1, :],
                    qa_ap[s * sc : (s + 1) * sc].rearrange("j p d -> p j d"),
                )

            for h in range(2):
                eng = ASSIGN[s][h]
                bs = slice(h * H, (h + 1) * H)
                if eng == "POOL":
                    # exp -> f32 staging; Pool normalize_recip divides by the
                    # DVE-computed rowsum and casts to bf16 in one Q7 op.
                    ef = stage.tile([128, H, 2, K], f32, name="ef")
                    nc.scalar.activation(ef[:], qe[:, bs, :, :], Exp,
                                         bias=zbias[:])
                    nc.vector.tensor_reduce(acc[:, bs, :], ef[:], X, Add)
                    for j in range(h * H, (h + 1) * H):
                        for t in range(2):
                            nc.gpsimd.normalize_recip(
                                ebf[:, j, t * K:(t + 1) * K],
                                ef[:, j - h * H, t, :],
                                acc[:, j, t:t + 1],
                            )
                        emit_chunk_matmuls(ebf[:, j, :], s * sc + j)
                else:
                    # exp -> bf16 in place of the matmul operand, in-place
                    # scale on DVE (fast) or ACT (uses its slack).
                    if fine:
                        for j in range(h * H, (h + 1) * H):
                            nc.scalar.activation(
                                ebf[:, j, :], qe[:, j:j + 1, :, :], Exp,
                                bias=zbias[:])
                            nc.vector.tensor_reduce(
                                acc[:, j, 0:1], ebf[:, j, 0:K], X, Add)
                            nc.vector.tensor_reduce(
                                acc[:, j, 1:2], ebf[:, j, K:2 * K], X, Add)
                            nc.vector.reciprocal(rt[:, j, :], acc[:, j, :])
                    else:
                        nc.scalar.activation(ebf[:, bs, :], qe[:, bs, :, :],
                                             Exp, bias=zbias[:])
                        nc.vector.tensor_reduce(
                            acc[:, bs, 0], ebf[:, bs, 0:K], X, Add)
                        nc.vector.tensor_reduce(
                            acc[:, bs, 1], ebf[:, bs, K:2 * K], X, Add)
                        nc.vector.reciprocal(rt[:, bs, :], acc[:, bs, :])
                    for j in range(h * H, (h + 1) * H):
                        # qa half first: the G_bb matmuls only need qa, so PE
                        # can start while the q half is still scaling.
                        if eng == "DVE":
                            nc.vector.tensor_scalar_mul(
                                ebf[:, j, K:2 * K], ebf[:, j, K:2 * K],
                                rt[:, j, 1:2])
                            nc.vector.tensor_scalar_mul(
                                ebf[:, j, 0:K], ebf[:, j, 0:K],
                                rt[:, j, 0:1])
                        else:
                            nc.scalar.mul(ebf[:, j, K:2 * K],
                                          ebf[:, j, K:2 * K], rt[:, j, 1:2])
                            nc.scalar.mul(ebf[:, j, 0:K],
                                          ebf[:, j, 0:K], rt[:, j, 0:1])
                        emit_chunk_matmuls(ebf[:, j, :], s * sc + j)

        # Epilogue: 10 x [128, 128] blocks packed as [128, 1280].
        ot = outp.tile([128, 10 * 128], f32, name="ot")
        nc.vector.tensor_copy(ot[:, 0:512], ps[0][:])
        nc.scalar.copy(ot[:, 512:896], ps[1][:])
        nc.vector.tensor_copy(ot[:, 896:1152], ps[2][:])
        nc.scalar.copy(ot[:, 1152:1280], ps[3][:])
        nc.sync.dma_start(out_ap[:], ot[:])

    nc.compile()
    return nc


def get_nc(shard_rows=SHARD):
    if shard_rows not in _CACHE:
        _CACHE[shard_rows] = _build(shard_rows)
    return _CACHE[shard_rows]


def finish_loss(partials_sum):
    """Host-side reduction: partials [128, 1280] float64 -> scalar loss."""
    P = partials_sum
    G_aa = np.empty((K, K))
    G_aa[0:128, :] = P[:, 0:256]
    G_aa[128:, 128:] = P[:, 512:640]
    G_aa[128:, 0:128] = P[:, 128:256].T          # = G_aa[0:128, 128:].T
    G_ab = np.empty((K, K))
    G_ab[0:128, :] = P[:, 256:512]
    G_ab[128:, :] = P[:, 640:896]
    G_bb = np.empty((K, K))
    G_bb[0:128, :] = P[:, 896:1152]
    G_bb[128:, 128:] = P[:, 1152:1280]
    G_bb[128:, 0:128] = P[:, 1024:1152].T        # = G_bb[0:128, 128:].T

    # Column marginals: softmax rows sum to 1 => colsum = row-sums of Gram.
    cs_q = G_aa.sum(axis=1)
    cs_qa = G_bb.sum(axis=1)
    p_q = cs_q / cs_q.sum()
    p_qa = cs_qa / cs_qa.sum()
    ne_loss = (p_q * np.log(p_q)).sum() + (p_qa * np.log(p_qa)).sum()

    na = np.maximum(np.sqrt(np.diag(G_aa)), EPS)
    nb = np.maximum(np.sqrt(np.diag(G_bb)), EPS)
    eye = np.eye(K)
    l_aa = G_aa / np.outer(na, na) - eye * LARGE_NUM
    l_bb = G_bb / np.outer(nb, nb) - eye * LARGE_NUM
    l_ab = G_ab / np.outer(na, nb)
    l_ba = l_ab.T

    def xent_mean(left, right):
        # rows: label k selects column k of the *left* block
        z = np.concatenate([left, right], axis=1)
        m = z.max(axis=1, keepdims=True)
        lse = np.log(np.exp(z - m).sum(axis=1)) + m[:, 0]
        return (lse - np.diag(left)).mean()

    loss_a = xent_mean(l_ab, l_aa)
    loss_b = xent_mean(l_ba, l_bb)
    return loss_a + loss_b + ne_loss


def kernel(q, q_a):
    from concourse import bass_utils

    q = np.ascontiguousarray(np.asarray(q, dtype=np.float32))
    q_a = np.ascontiguousarray(np.asarray(q_a, dtype=np.float32))
    assert q.shape == (N_TOTAL, K) and q_a.shape == (N_TOTAL, K)

    nc = get_nc()
    n_chunks = SHARD // CHUNK_P
    in_maps = [
        {
            "q": q[c * SHARD : (c + 1) * SHARD].reshape(n_chunks, CHUNK_P, K),
            "q_a": q_a[c * SHARD : (c + 1) * SHARD].reshape(n_chunks, CHUNK_P, K),
        }
        for c in range(N_CORES)
    ]
    global _LAST
    # Transient device flakes can corrupt a run (observed once: NaN output);
    # retry a couple of times on a non-finite result.
    for _attempt in range(3):
        res = bass_utils.run_bass_kernel_spmd(
            nc, in_maps, core_ids=list(range(N_CORES)), trace=_TRACE
        )
        _LAST = res
        total = np.zeros((CHUNK_P, 10 * 128), dtype=np.float64)
        for r in res.results:
            total += r["partials"].astype(np.float64)
        loss = finish_loss(total)
        if np.isfinite(loss):
            break
    return np.asarray(loss, dtype=np.float32).reshape(())


# revision 15
# speedup vs baseline: 1.1896x; 1.1896x over previous
"""Trainium2 Bass kernel for ClusterContrastiveLoss (N=65536, K=256).

Data-parallel over the batch axis: each of the 8 cores processes 8192 rows of
q/q_a, computing row-softmax and accumulating the K x K Gram matrices
    G_aa = qs^T @ qs,  G_ab = qs^T @ qas,  G_bb = qas^T @ qas
Since G_aa/G_bb are symmetric, only their upper block-triangles are computed
(4 matmuls per 128-row chunk with free dims 512/384/256/128 instead of
512/512/256/256).  Column marginals come for free on the host: softmax rows
sum to 1, so colsum(qs)[k] = sum_j G_aa[k, j].

The kernel is DMA-bound (16 MB/core of fp32 input ~= 50us at the ~340 GB/s
per-core HBM rate), so the softmax element work is spread over THREE engines,
each kept under that window.  Hardware-measured rates for the [128, 256]
per-(chunk, tensor) scale (multiply by a per-partition 1/rowsum):
    DVE  tensor_scalar (AP scalar)   281 ns   (2x_1p mode + Ptr overhead)
    ACT  scalar.mul    (AP scale)    583 ns
    Pool normalize_recip (custom Q7) 424 ns   (fused reciprocal + divide)
tensor_reduce only has a 1x uop (~1.07 ns/elem), so the rowsum reduce alone
costs ~35us of DVE -- hence the scale work mostly lives on Pool/ACT.
Per-half-superchunk (4 chunks) assignment:
    Pool halves: ACT exp -> f32 staging, DVE reduce, Pool normalize -> bf16
    DVE/ACT halves: ACT exp -> bf16, DVE reduce, DVE-or-ACT in-place scale
The first/last superchunks run chunk-at-a-time (fine DMA + B=1 exp) to
collapse the pipeline ramp/tail; dummy exp / normalize ops at kernel start
hoist the ACT table load (~1.3us) and the Q7 library load (~6us) under the
first DMA wait.  q DMAs issue from sync, q_a from gpsimd SWDGE so descriptor
issue never starves the 16 DMA engines.  Host sums per-core partials and
evaluates the closed-form loss on the tiny K x K matrices in float64.
"""

import numpy as np

N_TOTAL = 65536
K = 256
N_CORES = 8
SHARD = N_TOTAL // N_CORES  # 8192 rows per core
CHUNK_P = 128               # rows per compute chunk (SBUF partition dim)
SUPER = 8                   # chunks per DMA superchunk (1 MB per tensor)
EPS = 1e-8
LARGE_NUM = 1e9

_CACHE = {}

# Test-harness knobs (ignored in normal use): set _TRACE=True before calling
# kernel() to capture an NTFF profile; the BassKernelResults lands in _LAST.
_TRACE = False
_LAST = None

# Per-(superchunk, half) scale-engine assignment; tuned so each engine's
# total work stays under the ~50us DMA window (see module docstring).
ASSIGN = {
    0: ("DVE", "DVE"),
    1: ("POOL", "ACT"),
    2: ("POOL", "DVE"),
    3: ("POOL", "ACT"),
    4: ("POOL", "POOL"),
    5: ("POOL", "ACT"),
    6: ("POOL", "POOL"),
    7: ("POOL", "DVE"),
}


def _build(shard_rows):
    from contextlib import ExitStack

    import concourse.bass as bass  # noqa: F401
    import concourse.tile as tile
    from concourse import bacc, mybir

    n_chunks = shard_rows // CHUNK_P
    sc = min(SUPER, n_chunks)      # chunks per superchunk
    n_super = n_chunks // sc
    H = sc // 2                    # chunks per half-superchunk

    f32 = mybir.dt.float32
    bf16 = mybir.dt.bfloat16
    Exp = mybir.ActivationFunctionType.Exp
    Add = mybir.AluOpType.add
    X = mybir.AxisListType.X

    nc = bacc.Bacc("TRN2", target_bir_lowering=False, debug=False)
    q_ap = nc.dram_tensor(
        "q", [n_chunks, CHUNK_P, K], f32, kind="ExternalInput"
    ).ap()
    qa_ap = nc.dram_tensor(
        "q_a", [n_chunks, CHUNK_P, K], f32, kind="ExternalInput"
    ).ap()
    out_ap = nc.dram_tensor(
        "partials", [CHUNK_P, 10 * 128], f32, kind="ExternalOutput"
    ).ap()

    with tile.TileContext(nc) as tc, ExitStack() as ctx:
        inp = ctx.enter_context(tc.tile_pool(name="inp", bufs=5))
        work = ctx.enter_context(tc.tile_pool(name="work", bufs=4))
        stage = ctx.enter_context(tc.tile_pool(name="stage", bufs=3))
        stats = ctx.enter_context(tc.tile_pool(name="stats", bufs=4))
        psum = ctx.enter_context(tc.tile_pool(name="psum", bufs=1, space="PSUM"))
        outp = ctx.enter_context(tc.tile_pool(name="outp", bufs=1))

        # Accumulators, one PSUM bank each, live across the whole kernel.
        # Missing blocks are transposes of computed ones (host reconstructs):
        #   ps_a = [G_aa[0:128, :]   | G_ab[0:128, :]]   N=512
        #   ps_b = [G_aa[128:, 128:] | G_ab[128:, :]]    N=384
        #   ps_c =  G_bb[0:128, :]                       N=256
        #   ps_d =  G_bb[128:, 128:]                     N=128
        ps = [
            psum.tile([128, 512], f32, name="ps_a"),
            psum.tile([128, 384], f32, name="ps_b"),
            psum.tile([128, 256], f32, name="ps_c"),
            psum.tile([128, 128], f32, name="ps_d"),
        ]
        zbias = stats.tile([128, 1], f32, name="zbias", bufs=1)
        nc.vector.memset(zbias[:], 0.0)
        # Dummy ops: force the ACT table load (~1.3us) and the Pool Q7
        # library load (~6us) to happen under the first DMA wait instead of
        # stalling the first real activation / normalize.  warm_* tiles are
        # private so these never acquire waits that could head-of-line block
        # the strict-FIFO engine queues.
        warm_in = stats.tile([128, 1], f32, name="warm_in", bufs=1)
        warm_dn = stats.tile([128, 1], f32, name="warm_dn", bufs=1)
        nc.gpsimd.memset(warm_in[:], 0.0)
        nc.gpsimd.memset(warm_dn[:], 1.0)
        warm_p = stats.tile([128, 1], bf16, name="warm_p", bufs=1)
        nc.gpsimd.normalize_recip(warm_p[:], warm_in[:], warm_dn[:])
        warm_a = stats.tile([128, 1], bf16, name="warm_a", bufs=1)
        nc.scalar.activation(warm_a[:], zbias[:], Exp, bias=zbias[:])

        def emit_chunk_matmuls(rhs, it):
            first = it == 0
            last = it == n_chunks - 1
            nc.tensor.matmul(ps[2][:], rhs[:, 256:384], rhs[:, 256:512],
                             start=first, stop=last)
            nc.tensor.matmul(ps[3][:], rhs[:, 384:512], rhs[:, 384:512],
                             start=first, stop=last)
            nc.tensor.matmul(ps[0][:], rhs[:, 0:128], rhs[:, 0:512],
                             start=first, stop=last)
            nc.tensor.matmul(ps[1][:], rhs[:, 128:256], rhs[:, 128:512],
                             start=first, stop=last)

        for s in range(n_super):
            fine = s == 0 or s == n_super - 1
            # Interleaved layout: qe[:, j, 0, :] = q chunk, qe[:, j, 1, :] =
            # q_a chunk, so each chunk's scaled [qs | qas] in ebf is a
            # contiguous [128, 512] whose slices serve as both lhsT and rhs.
            qe = inp.tile([128, sc, 2, K], f32, name="qe")
            ebf = work.tile([128, sc, 2 * K], bf16, name="ebf")
            acc = stats.tile([128, sc, 2], f32, name="acc")
            rt = stats.tile([128, sc, 2], f32, name="rt")

            qa_pending = list(range(sc))

            def emit_qa(n, s=s, qe=qe, pending=qa_pending):
                # Fine superchunks: qa chunk DMAs trickle out of the scalar
                # queue a couple of chunks ahead of the exp that needs them.
                for _ in range(n):
                    if pending:
                        j = pending.pop(0)
                        nc.scalar.dma_start(
                            qe[:, j : j + 1, 1, :],
                            qa_ap[s * sc + j : s * sc + j + 1].rearrange(
                                "j p d -> p j d"),
                        )

            if fine:
                # Ramp/tail: per-chunk DMAs so the first exp starts after
                # ~256KB (not 2MB) and the tail dependency chain is short.
                # qa issues from the scalar queue (interleaved ahead of the
                # exps below) -- NEVER from Pool, whose strict FIFO must stay
                # clear for normalize_recip.
                for j in range(sc):
                    nc.sync.dma_start(
                        qe[:, j : j + 1, 0, :],
                        q_ap[s * sc + j : s * sc + j + 1].rearrange(
                            "j p d -> p j d"),
                    )
                emit_qa(2)
            else:
                nc.sync.dma_start(
                    qe[:, :, 0, :],
                    q_ap[s * sc : (s + 1) * sc].rearrange("j p d -> p j d"),
                )
                nc.sync.dma_start(
                    qe[:, :, 1, :],
                    qa_ap[s * sc : (s + 1) * sc].rearrange("j p d -> p j d"),
                )

            for h in range(2):
                eng = ASSIGN[s][h]
                bs = slice(h * H, (h + 1) * H)
                if eng == "POOL":
                    # exp -> f32 staging; Pool normalize_recip divides by the
                    # DVE-computed rowsum and casts to bf16 in one Q7 op.
                    ef = stage.tile([128, H, 2, K], f32, name="ef")
                    if fine:
                        for j in range(h * H, (h + 1) * H):
                            emit_qa(1)
                            jj = j - h * H
                            nc.scalar.activation(
                                ef[:, jj:jj + 1, :, :], qe[:, j:j + 1, :, :],
                                Exp, bias=zbias[:])
                            nc.vector.tensor_reduce(
                                acc[:, j:j + 1, :], ef[:, jj:jj + 1, :, :],
                                X, Add)
                    else:
                        nc.scalar.activation(ef[:], qe[:, bs, :, :], Exp,
                                             bias=zbias[:])
                        nc.vector.tensor_reduce(acc[:, bs, :], ef[:], X, Add)
                    for j in range(h * H, (h + 1) * H):
                        for t in range(2):
                            nc.gpsimd.normalize_recip(
                                ebf[:, j, t * K:(t + 1) * K],
                                ef[:, j - h * H, t, :],
                                acc[:, j, t:t + 1],
                            )
                        emit_chunk_matmuls(ebf[:, j, :], s * sc + j)
                else:
                    # exp -> bf16 in place of the matmul operand, in-place
                    # scale on DVE (fast) or ACT (uses its slack).
                    if fine:
                        for j in range(h * H, (h + 1) * H):
                            emit_qa(1)
                            nc.scalar.activation(
                                ebf[:, j, :], qe[:, j:j + 1, :, :], Exp,
                                bias=zbias[:])
                            nc.vector.tensor_reduce(
                                acc[:, j, 0:1], ebf[:, j, 0:K], X, Add)
                            nc.vector.tensor_reduce(
                                acc[:, j, 1:2], ebf[:, j, K:2 * K], X, Add)
                            nc.vector.reciprocal(rt[:, j, :], acc[:, j, :])
                    else:
                        nc.scalar.activation(ebf[:, bs, :], qe[:, bs, :, :],
                                             Exp, bias=zbias[:])
                        nc.vector.tensor_reduce(
                            acc[:, bs, 0], ebf[:, bs, 0:K], X, Add)
                        nc.vector.tensor_reduce(
                            acc[:, bs, 1], ebf[:, bs, K:2 * K], X, Add)
                        nc.vector.reciprocal(rt[:, bs, :], acc[:, bs, :])
                    for j in range(h * H, (h + 1) * H):
                        # qa half first: the G_bb matmuls only need qa, so PE
                        # can start while the q half is still scaling.
                        if eng == "DVE":
                            nc.vector.tensor_scalar_mul(
                                ebf[:, j, K:2 * K], ebf[:, j, K:2 * K],
                                rt[:, j, 1:2])
                            nc.vector.tensor_scalar_mul(
                                ebf[:, j, 0:K], ebf[:, j, 0:K],
                                rt[:, j, 0:1])
                        else:
                            nc.scalar.mul(ebf[:, j, K:2 * K],
                                          ebf[:, j, K:2 * K], rt[:, j, 1:2])
                            nc.scalar.mul(ebf[:, j, 0:K],
                                          ebf[:, j, 0:K], rt[:, j, 0:1])
                        emit_chunk_matmuls(ebf[:, j, :], s * sc + j)

        # Epilogue: 10 x [128, 128] blocks packed as [128, 1280].
        ot = outp.tile([128, 10 * 128], f32, name="ot")
        nc.vector.tensor_copy(ot[:, 0:512], ps[0][:])
        nc.scalar.copy(ot[:, 512:896], ps[1][:])
        nc.vector.tensor_copy(ot[:, 896:1152], ps[2][:])
        nc.scalar.copy(ot[:, 1152:1280], ps[3][:])
        nc.sync.dma_start(out_ap[:], ot[:])

    nc.compile()
    return nc


def get_nc(shard_rows=SHARD):
    if shard_rows not in _CACHE:
        _CACHE[shard_rows] = _build(shard_rows)
    return _CACHE[shard_rows]


def finish_loss(partials_sum):
    """Host-side reduction: partials [128, 1280] float64 -> scalar loss."""
    P = partials_sum
    G_aa = np.empty((K, K))
    G_aa[0:128, :] = P[:, 0:256]
    G_aa[128:, 128:] = P[:, 512:640]
    G_aa[128:, 0:128] = P[:, 128:256].T          # = G_aa[0:128, 128:].T
    G_ab = np.empty((K, K))
    G_ab[0:128, :] = P[:, 256:512]
    G_ab[128:, :] = P[:, 640:896]
    G_bb = np.empty((K, K))
    G_bb[0:128, :] = P[:, 896:1152]
    G_bb[128:, 128:] = P[:, 1152:1280]
    G_bb[128:, 0:128] = P[:, 1024:1152].T        # = G_bb[0:128, 128:].T

    # Column marginals: softmax rows sum to 1 => colsum = row-sums of Gram.
    cs_q = G_aa.sum(axis=1)
    cs_qa = G_bb.sum(axis=1)
    p_q = cs_q / cs_q.sum()
    p_qa = cs_qa / cs_qa.sum()
    ne_loss = (p_q * np.log(p_q)).sum() + (p_qa * np.log(p_qa)).sum()

    na = np.maximum(np.sqrt(np.diag(G_aa)), EPS)
    nb = np.maximum(np.sqrt(np.diag(G_bb)), EPS)
    eye = np.eye(K)
    l_aa = G_aa / np.outer(na, na) - eye * LARGE_NUM
    l_bb = G_bb / np.outer(nb, nb) - eye * LARGE_NUM
    l_ab = G_ab / np.outer(na, nb)
    l_ba = l_ab.T

    def xent_mean(left, right):
        # rows: label k selects column k of the *left* block
        z = np.concatenate([left, right], axis=1)
        m = z.max(axis=1, keepdims=True)
        lse = np.log(np.exp(z - m).sum(axis=1)) + m[:, 0]
        return (lse - np.diag(left)).mean()

    loss_a = xent_mean(l_ab, l_aa)
    loss_b = xent_mean(l_ba, l_bb)
    return loss_a + loss_b + ne_loss


def kernel(q, q_a):
    from concourse import bass_utils

    q = np.ascontiguousarray(np.asarray(q, dtype=np.float32))
    q_a = np.ascontiguousarray(np.asarray(q_a, dtype=np.float32))
    assert q.shape == (N_TOTAL, K) and q_a.shape == (N_TOTAL, K)

    nc = get_nc()
    n_chunks = SHARD // CHUNK_P
    in_maps = [
        {
            "q": q[c * SHARD : (c + 1) * SHARD].reshape(n_chunks, CHUNK_P, K),
            "q_a": q_a[c * SHARD : (c + 1) * SHARD].reshape(n_chunks, CHUNK_P, K),
        }
        for c in range(N_CORES)
    ]
    global _LAST
    # Transient device flakes can corrupt a run (observed once: NaN output);
    # retry a couple of times on a non-finite result.
    for _attempt in range(3):
        res = bass_utils.run_bass_kernel_spmd(
            nc, in_maps, core_ids=list(range(N_CORES)), trace=_TRACE
        )
        _LAST = res
        total = np.zeros((CHUNK_P, 10 * 128), dtype=np.float64)
        for r in res.results:
            total += r["partials"].astype(np.float64)
        loss = finish_loss(total)
        if np.isfinite(loss):
            break
    return np.asarray(loss, dtype=np.float32).reshape(())


# revision 16
# speedup vs baseline: 1.2567x; 1.0564x over previous
"""Trainium2 Bass kernel for ClusterContrastiveLoss (N=65536, K=256).

Data-parallel over the batch axis: each of the 8 cores processes 8192 rows of
q/q_a, computing row-softmax and accumulating the K x K Gram matrices
    G_aa = qs^T @ qs,  G_ab = qs^T @ qas,  G_bb = qas^T @ qas
Since G_aa/G_bb are symmetric, only their upper block-triangles are computed
(4 matmuls per 128-row sub-block with free dims 512/384/256/128 instead of
512/512/256/256).  Column marginals come for free on the host: softmax rows
sum to 1, so colsum(qs)[k] = sum_j G_aa[k, j].

Layout: PARTITION-MAJOR superchunks.  Each superchunk holds 1024 consecutive
batch rows as [128 partitions, 8 sub-rows, 256]: partition p owns rows
p*8..p*8+7, so every DMA descriptor is an 8 KB contiguous DRAM run (vs 1 KB
for the row-interleaved layout).  Descriptor issue drops ~4x (sync engine
~28us instead of ~47us saturated) and the 16 DMA engines stream near peak.
A "sub-row" (one partition-row slice [128, 2*256] of packed [qs|qas]) plays
the role the 128-row chunk played before; the Gram contraction is over the
128 partitions exactly as before.

The kernel is DMA-bound (16 MB/core of fp32 input ~= 48-54us), so the
softmax element work is spread over three engines, each under that window,
using hardware-measured per-[128,256]-op rates:
    DVE  tensor_scalar (AP scalar)    281 ns  (2x_1p + Ptr overhead)
    ACT  scalar.mul    (AP scale)     583 ns
    Pool normalize_recip (custom Q7)  424 ns  (fused reciprocal + divide,
         bf16 in/out verified exact vs f32 reference on HW)
tensor_reduce only has a 1x uop (~1.07 ns/elem), so rowsums use a bf16
pair-fold (tensor_tensor add at 2x) + half-width reduce where batched.
The first/last superchunks run sub-row-at-a-time (fine DMA + small exps) to
collapse the pipeline ramp/tail; dummy exp / normalize ops at kernel start
hoist the ACT table load (~1.3us) and the Pool Q7 library load (~6us) under
the first DMA wait.  Pool's strict FIFO carries ONLY normalize ops.
Host sums per-core partials and evaluates the closed-form loss on the tiny
K x K matrices in float64.
"""

import numpy as np

N_TOTAL = 65536
K = 256
N_CORES = 8
SHARD = N_TOTAL // N_CORES  # 8192 rows per core
R = 8                       # sub-rows per partition per superchunk
SROWS = 128 * R             # batch rows per superchunk (1024)
EPS = 1e-8
LARGE_NUM = 1e9

_CACHE = {}

# Test-harness knobs (ignored in normal use): set _TRACE=True before calling
# kernel() to capture an NTFF profile; the BassKernelResults lands in _LAST.
_TRACE = False
_LAST = None

# Per-(superchunk, half) scale-engine assignment; tuned so each engine's
# total work stays under the DMA window (see module docstring).
ASSIGN = {
    0: ("DVE", "DVE"),
    1: ("POOL", "POOL"),
    2: ("POOL", "ACT"),
    3: ("POOL", "POOL"),
    4: ("POOL", "DVE"),
    5: ("POOL", "POOL"),
    6: ("POOL", "POOL"),
    7: ("POOL", "DVE"),
}


def _build(shard_rows):
    from contextlib import ExitStack

    import concourse.bass as bass  # noqa: F401
    import concourse.tile as tile
    from concourse import bacc, bass_isa, mybir

    n_super = shard_rows // SROWS
    n_sub = n_super * R            # 64 sub-rows total
    H = R // 2                     # sub-rows per half-superchunk

    f32 = mybir.dt.float32
    bf16 = mybir.dt.bfloat16
    Exp = mybir.ActivationFunctionType.Exp
    Add = mybir.AluOpType.add
    X = mybir.AxisListType.X

    nc = bacc.Bacc("TRN2", target_bir_lowering=False, debug=False)
    q_ap = nc.dram_tensor(
        "q", [n_super, 128, R, K], f32, kind="ExternalInput"
    ).ap()
    qa_ap = nc.dram_tensor(
        "q_a", [n_super, 128, R, K], f32, kind="ExternalInput"
    ).ap()
    out_ap = nc.dram_tensor(
        "partials", [128, 10 * 128], f32, kind="ExternalOutput"
    ).ap()

    def nrecip(out, in_, denom):
        # normalize_recip with bf16 input: the Q7 read FIFO upconverts to
        # f32 (verified exact on HW); bass's wrapper asserts f32, so emit
        # the instruction directly.
        g = nc.gpsimd
        return g.add_instruction(bass_isa.InstNormalizeRecip(
            name=f"I-{nc.next_id()}",
            ins=[g.lower_ap(in_, for_isa=True),
                 g.lower_ap(denom, for_isa=True)],
            outs=[g.lower_ap(out, for_isa=True),
                  g.lower_ap(denom, for_isa=True)],
            _channels=in_.shape[0], _m_tile=in_.free_size(),
        ))

    with tile.TileContext(nc) as tc, ExitStack() as ctx:
        inp = ctx.enter_context(tc.tile_pool(name="inp", bufs=5))
        work = ctx.enter_context(tc.tile_pool(name="work", bufs=4))
        stats = ctx.enter_context(tc.tile_pool(name="stats", bufs=4))
        psum = ctx.enter_context(tc.tile_pool(name="psum", bufs=1, space="PSUM"))
        outp = ctx.enter_context(tc.tile_pool(name="outp", bufs=1))

        # Accumulators, one PSUM bank each, live across the whole kernel.
        # Missing blocks are transposes of computed ones (host reconstructs):
        #   ps_a = [G_aa[0:128, :]   | G_ab[0:128, :]]   N=512
        #   ps_b = [G_aa[128:, 128:] | G_ab[128:, :]]    N=384
        #   ps_c =  G_bb[0:128, :]                       N=256
        #   ps_d =  G_bb[128:, 128:]                     N=128
        ps = [
            psum.tile([128, 512], f32, name="ps_a"),
            psum.tile([128, 384], f32, name="ps_b"),
            psum.tile([128, 256], f32, name="ps_c"),
            psum.tile([128, 128], f32, name="ps_d"),
        ]
        zbias = stats.tile([128, 1], f32, name="zbias", bufs=1)
        nc.vector.memset(zbias[:], 0.0)
        # Dummy ops with private tiles: force the ACT table load (~1.3us)
        # and the Pool Q7 library load (~6us) under the first DMA wait.
        warm_in = stats.tile([128, 1], bf16, name="warm_in", bufs=1)
        warm_dn = stats.tile([128, 1], f32, name="warm_dn", bufs=1)
        nc.gpsimd.memset(warm_in[:], 1.0)
        nc.gpsimd.memset(warm_dn[:], 1.0)
        warm_p = stats.tile([128, 1], bf16, name="warm_p", bufs=1)
        nrecip(warm_p[:], warm_in[:], warm_dn[:])
        warm_a = stats.tile([128, 1], bf16, name="warm_a", bufs=1)
        nc.scalar.activation(warm_a[:], zbias[:], Exp, bias=zbias[:])

        def emit_subrow_matmuls(rhs, it):
            first = it == 0
            last = it == n_sub - 1
            nc.tensor.matmul(ps[2][:], rhs[:, 256:384], rhs[:, 256:512],
                             start=first, stop=last)
            nc.tensor.matmul(ps[3][:], rhs[:, 384:512], rhs[:, 384:512],
                             start=first, stop=last)
            nc.tensor.matmul(ps[0][:], rhs[:, 0:128], rhs[:, 0:512],
                             start=first, stop=last)
            nc.tensor.matmul(ps[1][:], rhs[:, 128:256], rhs[:, 128:512],
                             start=first, stop=last)

        for s in range(n_super):
            fine = s == 0 or s == n_super - 1
            # qc[:, 0, :, :] <- q superchunk, qc[:, 1, :, :] <- q_a: each is a
            # contiguous 8KB/partition DMA.  ebf[:, j, :] is sub-row j's
            # packed [qs | qas] [128, 512] whose slices serve lhsT and rhs.
            qc = inp.tile([128, 2, R, K], f32, name="qc")
            ebf = work.tile([128, R, 2 * K], bf16, name="ebf")
            acc = stats.tile([128, R, 2], f32, name="acc")
            rt = stats.tile([128, R, 2], f32, name="rt")

            qsrc = q_ap[s:s + 1].rearrange("s p r d -> p s r d")
            qasrc = qa_ap[s:s + 1].rearrange("s p r d -> p s r d")
            if fine:
                # Ramp/tail: per-sub-row DMAs so the first exp starts after
                # ~256KB (not 2MB) and the tail dependency chain is short.
                for j in range(R):
                    nc.sync.dma_start(qc[:, 0:1, j:j + 1, :],
                                      qsrc[:, :, j:j + 1, :])
                    nc.sync.dma_start(qc[:, 1:2, j:j + 1, :],
                                      qasrc[:, :, j:j + 1, :])
            else:
                nc.sync.dma_start(qc[:, 0:1, :, :], qsrc)
                nc.sync.dma_start(qc[:, 1:2, :, :], qasrc)

            for h in range(2):
                eng = ASSIGN[s][h]
                bs = slice(h * H, (h + 1) * H)
                sub_fine = (s == 0 and h == 0) or (s == n_super - 1 and h == 1)
                if sub_fine:
                    # One [128, 512] exp per sub-row: input (t, d) iteration
                    # order equals the packed [qs | qas] output layout.
                    for j in range(h * H, (h + 1) * H):
                        nc.scalar.activation(ebf[:, j, :], qc[:, :, j, :],
                                             Exp, bias=zbias[:])
                        nc.vector.tensor_reduce(
                            acc[:, j, 0:1], ebf[:, j, 0:K], X, Add)
                        nc.vector.tensor_reduce(
                            acc[:, j, 1:2], ebf[:, j, K:2 * K], X, Add)
                        if eng != "POOL":
                            nc.vector.reciprocal(rt[:, j, :], acc[:, j, :])
                else:
                    # Two exps per half (one per tensor, FD=1024): strided
                    # [R/2, 256] output slices of the packed layout.
                    nc.scalar.activation(ebf[:, bs, 0:K], qc[:, 0, bs, :],
                                         Exp, bias=zbias[:])
                    nc.scalar.activation(ebf[:, bs, K:2 * K], qc[:, 1, bs, :],
                                         Exp, bias=zbias[:])
                    # Rowsums: bf16 pair-fold at 2x then half-width reduce
                    # (tensor_reduce alone is 1x and would be ~35us/core).
                    fw = stats.tile([128, H, 2, K // 2], bf16, name="fw")
                    hv = ebf[:, bs, :].rearrange("p r (t x) -> p r t x", t=2)
                    nc.vector.tensor_tensor(
                        fw[:], hv[:, :, :, 0:K // 2], hv[:, :, :, K // 2:K],
                        Add)
                    nc.vector.tensor_reduce(acc[:, bs, :], fw[:], X, Add)
                    if eng != "POOL":
                        nc.vector.reciprocal(rt[:, bs, :], acc[:, bs, :])
                for j in range(h * H, (h + 1) * H):
                    # qa half first: the G_bb matmuls only need qa, so PE
                    # can start while the q half is still scaling.
                    if eng == "POOL":
                        nrecip(ebf[:, j, K:2 * K], ebf[:, j, K:2 * K],
                               acc[:, j, 1:2])
                        nrecip(ebf[:, j, 0:K], ebf[:, j, 0:K],
                               acc[:, j, 0:1])
                    elif eng == "DVE":
                        nc.vector.tensor_scalar_mul(
                            ebf[:, j, K:2 * K], ebf[:, j, K:2 * K],
                            rt[:, j, 1:2])
                        nc.vector.tensor_scalar_mul(
                            ebf[:, j, 0:K], ebf[:, j, 0:K], rt[:, j, 0:1])
                    else:
                        nc.scalar.mul(ebf[:, j, K:2 * K], ebf[:, j, K:2 * K],
                                      rt[:, j, 1:2])
                        nc.scalar.mul(ebf[:, j, 0:K], ebf[:, j, 0:K],
                                      rt[:, j, 0:1])
                    emit_subrow_matmuls(ebf[:, j, :], s * R + j)

        # Epilogue: 10 x [128, 128] blocks packed as [128, 1280].
        ot = outp.tile([128, 10 * 128], f32, name="ot")
        nc.vector.tensor_copy(ot[:, 0:512], ps[0][:])
        nc.scalar.copy(ot[:, 512:896], ps[1][:])
        nc.vector.tensor_copy(ot[:, 896:1152], ps[2][:])
        nc.scalar.copy(ot[:, 1152:1280], ps[3][:])
        nc.sync.dma_start(out_ap[:], ot[:])

    nc.compile()
    return nc


def get_nc(shard_rows=SHARD):
    if shard_rows not in _CACHE:
        _CACHE[shard_rows] = _build(shard_rows)
    return _CACHE[shard_rows]


def finish_loss(partials_sum):
    """Host-side reduction: partials [128, 1280] float64 -> scalar loss."""
    P = partials_sum
    G_aa = np.empty((K, K))
    G_aa[0:128, :] = P[:, 0:256]
    G_aa[128:, 128:] = P[:, 512:640]
    G_aa[128:, 0:128] = P[:, 128:256].T          # = G_aa[0:128, 128:].T
    G_ab = np.empty((K, K))
    G_ab[0:128, :] = P[:, 256:512]
    G_ab[128:, :] = P[:, 640:896]
    G_bb = np.empty((K, K))
    G_bb[0:128, :] = P[:, 896:1152]
    G_bb[128:, 128:] = P[:, 1152:1280]
    G_bb[128:, 0:128] = P[:, 1024:1152].T        # = G_bb[0:128, 128:].T

    # Column marginals: softmax rows sum to 1 => colsum = row-sums of Gram.
    cs_q = G_aa.sum(axis=1)
    cs_qa = G_bb.sum(axis=1)
    p_q = cs_q / cs_q.sum()
    p_qa = cs_qa / cs_qa.sum()
    ne_loss = (p_q * np.log(p_q)).sum() + (p_qa * np.log(p_qa)).sum()

    na = np.maximum(np.sqrt(np.diag(G_aa)), EPS)
    nb = np.maximum(np.sqrt(np.diag(G_bb)), EPS)
    eye = np.eye(K)
    l_aa = G_aa / np.outer(na, na) - eye * LARGE_NUM
    l_bb = G_bb / np.outer(nb, nb) - eye * LARGE_NUM
    l_ab = G_ab / np.outer(na, nb)
    l_ba = l_ab.T

    def xent_mean(left, right):
        # rows: label k selects column k of the *left* block
        z = np.concatenate([left, right], axis=1)
        m = z.max(axis=1, keepdims=True)
        lse = np.log(np.exp(z - m).sum(axis=1)) + m[:, 0]
        return (lse - np.diag(left)).mean()

    loss_a = xent_mean(l_ab, l_aa)
    loss_b = xent_mean(l_ba, l_bb)
    return loss_a + loss_b + ne_loss


def kernel(q, q_a):
    from concourse import bass_utils

    q = np.ascontiguousarray(np.asarray(q, dtype=np.float32))
    q_a = np.ascontiguousarray(np.asarray(q_a, dtype=np.float32))
    assert q.shape == (N_TOTAL, K) and q_a.shape == (N_TOTAL, K)

    nc = get_nc()
    n_super = SHARD // SROWS
    in_maps = [
        {
            # partition-major: superchunk s, partition p holds rows p*8..p*8+7
            "q": q[c * SHARD : (c + 1) * SHARD].reshape(n_super, 128, R, K),
            "q_a": q_a[c * SHARD : (c + 1) * SHARD].reshape(n_super, 128, R, K),
        }
        for c in range(N_CORES)
    ]
    global _LAST
    # Transient device flakes can corrupt a run (observed once: NaN output);
    # retry a couple of times on a non-finite result.
    for _attempt in range(3):
        res = bass_utils.run_bass_kernel_spmd(
            nc, in_maps, core_ids=list(range(N_CORES)), trace=_TRACE
        )
        _LAST = res
        total = np.zeros((128, 10 * 128), dtype=np.float64)
        for r in res.results:
            total += r["partials"].astype(np.float64)
        loss = finish_loss(total)
        if np.isfinite(loss):
            break
    return np.asarray(loss, dtype=np.float32).reshape(())
